# revision 46
# baseline (speedup 1.0000x reference)
"""Trainium2 Bass kernel for a causal attention block (QKV + RoPE + attention + out-proj).

Sharding over 8 NeuronCores: data-parallel over batch (4) x Megatron tensor-
parallel over heads (2 ranks x 8 heads). Each core computes a full-output
partial for its batch; host sums the two rank partials per batch.

v2 design (single-pass pipelined, bf16 data path):
  - All SBUF data is bf16 (PSUM stays fp32): DVE elementwise ops run in 2x
    mode, weight loads get FWL (128-col bf16 stationaries), DMA halves.
  - One fused schedule: token-block tb loop projects q/k/v for tb and
    interleaves the ACT-heavy attention of query-block tb-? between PE work,
    so exp overlaps projections instead of running as a separate phase.
  - Scores for the two heads of a pair are issued back-to-back with base
    partitions 0/64 -> the PE runs them concurrently (row-group tiling).
  - The v stationary carries a 64-wide ones block (M=128): the AV matmul
    produces softmax denominators replicated on psum partitions 64:127 for
    free, so normalization is a [64,512] reciprocal + one multiply.
  - One exp per (pair, kt-pair) over a [128,2048] psum tile (4 banks).
  - q/k head features are pre-permuted on the host ([even|odd] per head) so
    interleaved RoPE becomes rotate-half. Softmax skips max-subtraction
    (|s|/8 < 40 for this distribution, safe in fp32/bf16 exponent range).
"""
import numpy as np

B, T, D = 4, 2048, 1024
H_TOTAL, HD = 16, 64
N_CORES = 8
H = H_TOTAL // 2        # heads per core (TP rank)
NP = H // 2             # head pairs per core
FS = H * HD             # 512 sharded q/k/v features per core
DC = D // 128           # 8 d_model chunks
TB = T // 512           # 4 token blocks (query blocks)
TC = T // 128           # 16 token chunks (key tiles)
SCALE = 1.0 / np.sqrt(HD)

_CACHE = {}


def _split_waits(nc, mybir, maxw=1):
    """This env's walrus encodes at most one sem wait per instruction; move
    extra waits onto same-engine NoOp carriers inserted just before."""
    import copy
    eng_map = {
        mybir.EngineType.PE: nc.tensor,
        mybir.EngineType.DVE: nc.vector,
        mybir.EngineType.Activation: nc.scalar,
        mybir.EngineType.Pool: nc.gpsimd,
        mybir.EngineType.SP: nc.sync,
    }
    protos = {}

    def proto(engine):
        if engine not in protos:
            mi = eng_map[engine].nop(nofuse=True).ins
            for blk in nc.m.functions[0].blocks:
                insts = list(blk.instructions)
                if insts and insts[-1].name == mi.name:
                    blk.instructions = insts[:-1]
                    break
            protos[engine] = mi
        return protos[engine]

    ctr = 0
    for blk in nc.m.functions[0].blocks:
        out = []
        changed = False
        for inst in blk.instructions:
            si = inst.sync_info
            waits = list(si.on_wait) if si and si.on_wait else []
            if len(waits) > maxw and getattr(inst, "engine", None) is not None:
                head, keep = waits[:-maxw], waits[-maxw:]
                for i in range(0, len(head), maxw):
                    nop = copy.deepcopy(proto(inst.engine))
                    ctr += 1
                    nop.name = f"I-wsplit-{ctr}"
                    nop.sync_info = mybir.SyncInfo(on_wait=head[i:i + maxw], on_update=[])
                    out.append(nop)
                si.on_wait = keep
                changed = True
            out.append(inst)
        if changed:
            blk.instructions = out
    return nc


def _build_nc(R=1, attn=True):
    import concourse.bass as bass
    import concourse.mybir as mybir
    import concourse.tile as tile

    f32 = mybir.dt.float32
    bf16 = mybir.dt.bfloat16
    Exp = mybir.ActivationFunctionType.Exp

    nc = bass.Bass("TRN2", target_bir_lowering=False, debug=False)
    xT = nc.dram_tensor("xT", [D, T], bf16, kind="ExternalInput").ap()
    wqT = nc.dram_tensor("wqT", [D, FS], bf16, kind="ExternalInput").ap()
    wkT = nc.dram_tensor("wkT", [D, FS], bf16, kind="ExternalInput").ap()
    wvT = nc.dram_tensor("wvT", [D, FS], bf16, kind="ExternalInput").ap()
    woT = nc.dram_tensor("woT", [FS, D], bf16, kind="ExternalInput").ap()
    ropeC = nc.dram_tensor("ropeC", [128, T], bf16, kind="ExternalInput").ap()
    ropeS = nc.dram_tensor("ropeS", [128, T], bf16, kind="ExternalInput").ap()
    masks = nc.dram_tensor("masks", [4, 128, 512], bf16, kind="ExternalInput").ap()
    out = nc.dram_tensor("out", [T, D], f32, kind="ExternalOutput").ap()

    with tile.TileContext(nc) as tc:
      for _rep in range(R):
        with tc.tile_pool(name="persist", bufs=1) as persist, \
             tc.tile_pool(name="pp", bufs=2, space="PSUM") as pp, \
             tc.tile_pool(name="pss", bufs=2, space="PSUM") as pss, \
             tc.tile_pool(name="psc", bufs=1, space="PSUM") as psc, \
             tc.tile_pool(name="attn", bufs=6) as attnp, \
             tc.tile_pool(name="ptmp", bufs=2) as ptmp, \
             tc.tile_pool(name="nrm", bufs=2) as nrm, \
             tc.tile_pool(name="obuf", bufs=4) as obuf:
            xT_s = persist.tile([128, DC, T], bf16)
            qT = persist.tile([128, NP, T], bf16)
            kT = persist.tile([128, NP, T], bf16)
            vON = persist.tile([128, TC, H, 128], bf16)  # [tok, kt, h, v|ones]
            ctxT = persist.tile([128, NP, T], bf16)
            wqs = persist.tile([128, DC, FS], bf16)
            wks = persist.tile([128, DC, FS], bf16)
            wvs = persist.tile([128, DC, FS], bf16)
            wos = persist.tile([128, NP, D], bf16)
            rc = persist.tile([128, T], bf16)
            rs = persist.tile([128, T], bf16)
            mk = persist.tile([128, 4, 512], bf16)

            xr = xT.rearrange("(c p) t -> p c t", p=128)
            nc.sync.dma_start(xT_s[:, :, 0:512], xr[:, :, 0:512])
            nc.sync.dma_start(wks, wkT.rearrange("(c p) m -> p c m", p=128))
            nc.sync.dma_start(wqs, wqT.rearrange("(c p) m -> p c m", p=128))
            nc.sync.dma_start(rc, ropeC)
            nc.sync.dma_start(rs, ropeS)
            nc.sync.dma_start(wvs, wvT.rearrange("(c p) m -> p c m", p=128))
            for tb in range(1, TB):
                nc.sync.dma_start(
                    xT_s[:, :, tb * 512:(tb + 1) * 512],
                    xr[:, :, tb * 512:(tb + 1) * 512],
                )
            nc.sync.dma_start(mk, masks.rearrange("j p f -> p j f"))
            nc.sync.dma_start(wos, woT.rearrange("(c p) o -> p c o", p=128))
            # ones block of the AV stationary (denominator trick)
            nc.gpsimd.memset(vON[:, :, :, HD:128], 1.0)
            if attn in ("noact", "expfree"):
                at_dummy = persist.tile([128, 1024], bf16)
                nc.vector.memset(at_dummy, 0.001)
                nc.vector.memset(ctxT[:, :, :], 0.5)
            # dedicated at-tiles for diagonal kt tiles of band-offset j: the
            # trimmed exp never writes [0:128j) per head, so one startup
            # memset keeps those always-masked bands zero forever
            at_diag = {}
            for dj in (1, 2, 3):
                at_d = persist.tile([128, 1024], bf16, name=f"at_d{dj}")
                nc.vector.memset(at_d, 0.0)
                at_diag[dj] = at_d

            def proj_group(dst, w_sb, pair, tb):
                tsl = slice(tb * 512, (tb + 1) * 512)
                psum = pp.tile([128, 512], f32, tag="p512")
                for kc in range(DC):
                    nc.tensor.matmul(
                        psum,
                        w_sb[:, kc, pair * 128:(pair + 1) * 128],
                        xT_s[:, kc, tsl],
                        start=(kc == 0), stop=(kc == DC - 1),
                    )
                dsl = dst[:, pair, tsl]
                # DVE evict: keeps the ACT stream pure-exp during attention
                nc.vector.tensor_copy(out=dsl, in_=psum)
                # rotate-half rope; in-place cos-mul after the shifted
                # sin-muls read the raw values (WAR dep).
                tmp = ptmp.tile([128, 512], bf16, tag="ropetmp")
                for eng, hb in ((nc.vector, 0), (nc.gpsimd, 64)):
                    eng.tensor_mul(
                        out=tmp[hb:hb + 32, :],
                        in0=dsl[hb + 32:hb + 64],
                        in1=rs[hb + 32:hb + 64, tsl],
                    )
                    eng.tensor_mul(
                        out=tmp[hb + 32:hb + 64, :],
                        in0=dsl[hb:hb + 32],
                        in1=rs[hb:hb + 32, tsl],
                    )
                nc.vector.tensor_mul(out=dsl, in0=dsl, in1=rc[:, tsl])
                nc.vector.tensor_add(out=dsl, in0=dsl, in1=tmp)

            def v_group(tcv):
                psum = pp.tile([128, 512], f32, tag="p512")
                for kc in range(DC):
                    nc.tensor.matmul(
                        psum,
                        xT_s[:, kc, tcv * 128:(tcv + 1) * 128],
                        wvs[:, kc, :],
                        start=(kc == 0), stop=(kc == DC - 1),
                    )
                nc.vector.tensor_copy(
                    out=vON[:, tcv, :, 0:HD],
                    in_=psum.rearrange("p (h e) -> p h e", e=HD),
                )

            def outproj_chunk(qb, i):
                # both 512-wide output halves of one token chunk: the second
                # matmul of each fc reuses the loaded stationary (no LDWEIGHTS)
                tco = 4 * qb + i
                psA = pp.tile([128, 512], f32, tag="p512")
                psB = pp.tile([128, 512], f32, tag="p512")
                for fc in range(NP):
                    for ps, osl in ((psA, slice(0, 512)), (psB, slice(512, 1024))):
                        mi = nc.tensor.matmul(
                            ps,
                            ctxT[:, fc, tco * 128:(tco + 1) * 128],
                            wos[:, fc, osl],
                            start=(fc == 0), stop=(fc == NP - 1),
                        )
                        if osl.start:
                            mi.ins.ldweights = False
                for ps, osl in ((psA, slice(0, 512)), (psB, slice(512, 1024))):
                    ot = obuf.tile([128, 512], f32, tag="ot")
                    nc.vector.tensor_copy(out=ot, in_=ps)
                    nc.sync.dma_start(out[tco * 128:(tco + 1) * 128, osl], ot)

            def tb_groups(tb):
                gs = []
                for pair in range(NP):
                    gs.append(lambda p=pair, t=tb: proj_group(kT, wks, p, t))
                for pair in range(NP):
                    gs.append(lambda p=pair, t=tb: proj_group(qT, wqs, p, t))
                for tcv in range(4 * tb, 4 * tb + 4):
                    gs.append(lambda c=tcv: v_group(c))
                return gs

            filler = []

            def emit_filler(n):
                for _ in range(n):
                    if filler:
                        filler.pop(0)()

            # startup: project everything for tb=0 directly
            for g in tb_groups(0):
                g()

            if not attn:
                # timing-bisect mode: skip attention, fake ctxT
                nc.vector.memset(ctxT[:, :, :], 0.5)
                for tb in range(1, TB):
                    for g in tb_groups(tb):
                        g()
                for qb in range(TB):
                    for i in range(4):
                        outproj_chunk(qb, i)

            for qb in range(TB if attn else 0):
                if qb + 1 < TB:
                    filler.extend(tb_groups(qb + 1))
                if qb > 0:
                    filler.extend(
                        lambda q=qb - 1, j=i: outproj_chunk(q, j) for i in range(4)
                    )
                qsl = slice(qb * 512, (qb + 1) * 512)
                nkt = 4 * qb + 4
                for pair in range(NP):
                    pctx = {}
                    for hi in (0, 1):
                        pctx_h = psc.tile([128, 512], f32, tag=f"pctx{hi}")
                        pctx[hi] = pctx_h

                    def do_mask(kt, at):
                        # causal mask varies only in the 128-wide diagonal
                        # strip; [0:128j) was zeroed on Pool
                        j = kt - 4 * qb
                        if j < 0 or attn == "noact":
                            return
                        for hi in (0, 1):
                            st = hi * 512 + 128 * j
                            nc.vector.tensor_mul(
                                out=at[:, st:st + 128], in0=at[:, st:st + 128],
                                in1=mk[:, j, 128 * j:128 * j + 128],
                            )

                    def do_av(kt, at):
                        av_src = at_dummy if attn == "expfree" else at
                        for hi in (0, 1):
                            sl = slice(hi * 512, (hi + 1) * 512)
                            nc.tensor.matmul(
                                pctx[hi],
                                vON[:, kt, 2 * pair + hi, :],
                                av_src[:, sl],
                                start=(kt == 0), stop=(kt == nkt - 1),
                            )

                    pend = []
                    for kt in range(nkt):
                        ps = pss.tile([128, 1024], f32, tag="ps")
                        j = kt - 4 * qb
                        if 1 <= j <= 3 and attn != "noact":
                            at = at_diag[j]
                        else:
                            at = attnp.tile([128, 1024], bf16, tag="at")
                        if attn == "noact":
                            at = at_dummy
                        for hi, po in ((0, 0), (1, 64)):
                            nc.tensor.matmul(
                                ps[:, hi * 512:(hi + 1) * 512],
                                kT[po:po + 64, pair, kt * 128:(kt + 1) * 128],
                                qT[po:po + 64, pair, qsl],
                                start=True, stop=True,
                            )
                        # diagonal tiles: only columns f >= 128j can be
                        # unmasked; exp just that band ([0:128j) of the
                        # dedicated at_diag tile is permanently zero)
                        if attn == "noact":
                            pass
                        elif j > 0:
                            nc.scalar.activation(
                                at.rearrange("p (h f) -> p h f", h=2)[:, :, 128 * j:],
                                ps.rearrange("p (h f) -> p h f", h=2)[:, :, 128 * j:],
                                Exp, scale=float(SCALE),
                            )
                        else:
                            nc.scalar.activation(at, ps, Exp, scale=float(SCALE))
                        # mask lags exp by 1 unit, AV by 2: the mask gets a
                        # full unit-period of DVE queue turn before its AV,
                        # and `at` is ready when the in-order PE queue
                        # reaches the AV matmuls
                        pend.append((kt, at))
                        if len(pend) >= 2:
                            do_mask(*pend[-2])
                        if len(pend) >= 3:
                            do_av(*pend.pop(0))
                        if kt % 2 == 1:
                            emit_filler(1)
                    do_mask(*pend[-1])
                    for p in pend:
                        do_av(*p)
                    pend.clear()
                    for hi, po in ((0, 0), (1, 64)):
                        if attn == "noact":
                            continue
                        rinv = nrm.tile([64, 512], f32, tag=f"rinv{hi}")
                        with nc.allow_low_precision(reason="softmax denominator"):
                            nc.vector.reciprocal(rinv, pctx[hi][HD:128, :])
                        nc.vector.tensor_mul(
                            out=ctxT[po:po + 64, pair, qsl],
                            in0=pctx[hi][0:HD, :],
                            in1=rinv,
                        )
                emit_filler(len(filler))
            if attn:
                for i in range(4):
                    outproj_chunk(TB - 1, i)

    _split_waits(nc, mybir)
    return nc


def _make_runner(nc, n_cores):
    """Build the shard_map-jitted PJRT executable once (reusable across calls)."""
    import jax
    import concourse.mybir as mybir
    from jax.sharding import Mesh, PartitionSpec
    from jax.experimental.shard_map import shard_map
    from concourse import bass2jax as b2j

    b2j.install_neuronx_cc_hook()
    partition_name = nc.partition_id_tensor.name if nc.partition_id_tensor else None
    in_names, out_names, out_avals = [], [], []
    for alloc in nc.m.functions[0].allocations:
        if not isinstance(alloc, mybir.MemoryLocationSet):
            continue
        name = alloc.memorylocations[0].name
        if alloc.kind == "ExternalInput":
            if name != partition_name:
                in_names.append(name)
        elif alloc.kind == "ExternalOutput":
            out_names.append(name)
            out_avals.append(
                jax.core.ShapedArray(tuple(alloc.tensor_shape), mybir.dt.np(alloc.dtype))
            )
    all_in_names = list(in_names) + list(out_names)
    if partition_name is not None:
        all_in_names.append(partition_name)

    def _body(*args):
        operands = list(args)
        if partition_name is not None:
            operands.append(b2j.partition_id_tensor())
        return tuple(b2j._bass_exec_p.bind(
            *operands,
            out_avals=tuple(out_avals),
            in_names=tuple(all_in_names),
            out_names=tuple(out_names),
            lowering_input_output_aliases=(),
            sim_require_finite=True,
            sim_require_nnan=True,
            nc=nc,
        ))

    devices = jax.devices()[:n_cores]
    mesh = Mesh(np.asarray(devices), ("core",))
    n_in = len(in_names) + len(out_names)
    fn = jax.jit(
        shard_map(
            _body, mesh=mesh,
            in_specs=(PartitionSpec("core"),) * n_in,
            out_specs=(PartitionSpec("core"),) * len(out_names),
            check_rep=False,
        ),
        keep_unused=True,
    )



    def stage(in_maps):
        import jax as _jax
        per_core = [[np.asarray(m[name]) for name in in_names] for m in in_maps]
        concat_in = [
            np.concatenate([per_core[c][i] for c in range(n_cores)], axis=0)
            for i in range(len(in_names))
        ]
        concat_zeros = [
            np.zeros((n_cores * a.shape[0], *a.shape[1:]), a.dtype) for a in out_avals
        ]
        return [_jax.device_put(a) for a in concat_in + concat_zeros]

    def call_staged(staged):
        import jax as _jax
        out_arrs = fn(*staged)
        _jax.block_until_ready(out_arrs)
        return out_arrs

    def time_staged(staged):
        return call_staged(staged)

    def call(in_maps):
        import jax as _jax
        out_arrs = call_staged(stage(in_maps))
        return [
            {name: np.asarray(out_arrs[i]).reshape(n_cores, *out_avals[i].shape)[c]
             for i, name in enumerate(out_names)}
            for c in range(n_cores)
        ]

    call.stage = stage
    call.call_staged = call_staged
    call.time_staged = time_staged
    return call


def _host_tables():
    import ml_dtypes
    bf16 = ml_dtypes.bfloat16
    # rope tables in the permuted ([even dims | odd dims] per head) layout:
    # rows 0:32 -> freq j (x1 of head A), 32:64 -> freq j (x2 of head A), repeat.
    j = np.arange(32, dtype=np.float64)
    inv = 1.0 / (10000.0 ** (2.0 * j / HD))
    t = np.arange(T, dtype=np.float64)
    ang = np.outer(inv, t)                      # [32, T]
    c32 = np.cos(ang).astype(np.float32)
    s32 = np.sin(ang).astype(np.float32)
    ropeC = np.concatenate([c32, c32, c32, c32], axis=0).astype(bf16)    # [128, T]
    ropeS = np.concatenate([s32, -s32, s32, -s32], axis=0).astype(bf16)  # [128, T]
    # causal masks for diagonal tiles: mask[j][p, f] = 1 if p <= f - 128j
    p = np.arange(128)[:, None]
    f = np.arange(512)[None, :]
    masks = np.stack(
        [(p <= f - 128 * jj).astype(np.float32) for jj in range(4)], axis=0
    ).astype(bf16)
    return ropeC, ropeS, masks


def _perm_rows():
    # per head: [even dims, odd dims]
    perm = []
    for h in range(H):
        base = h * HD
        perm.extend(base + np.arange(0, HD, 2))
        perm.extend(base + np.arange(1, HD, 2))
    return np.asarray(perm)


def _prep_in_maps(x, Wq, Wk, Wv, Wo):
    import ml_dtypes
    bf16 = ml_dtypes.bfloat16
    ropeC, ropeS, masks = _host_tables()
    perm = _perm_rows()
    in_maps = []
    for c in range(N_CORES):
        b, r = c // 2, c % 2
        rows = slice(r * FS, (r + 1) * FS)
        in_maps.append({
            "xT": np.ascontiguousarray(x[b].T).astype(bf16),
            "wqT": np.ascontiguousarray(Wq[rows][perm].T).astype(bf16),
            "wkT": np.ascontiguousarray(Wk[rows][perm].T).astype(bf16),
            "wvT": np.ascontiguousarray(Wv[rows].T).astype(bf16),
            "woT": np.ascontiguousarray(Wo[:, rows].T).astype(bf16),
            "ropeC": ropeC,
            "ropeS": ropeS,
            "masks": masks,
        })
    return in_maps


def kernel(x, Wq, Wk, Wv, Wo):
    x = np.asarray(x, dtype=np.float32)
    Wq = np.asarray(Wq, dtype=np.float32)
    Wk = np.asarray(Wk, dtype=np.float32)
    Wv = np.asarray(Wv, dtype=np.float32)
    Wo = np.asarray(Wo, dtype=np.float32)

    if "runner" not in _CACHE:
        nc = _build_nc()
        _CACHE["runner"] = _make_runner(nc, N_CORES)
    call = _CACHE["runner"]

    results = call(_prep_in_maps(x, Wq, Wk, Wv, Wo))
    out = np.empty((B, T, D), dtype=np.float32)
    for b in range(B):
        out[b] = results[2 * b]["out"] + results[2 * b + 1]["out"]
    return out


# revision 49
# speedup vs baseline: 1.0569x; 1.0569x over previous
"""Trainium2 Bass kernel for a causal attention block (QKV + RoPE + attention + out-proj).

Sharding over 8 NeuronCores: data-parallel over batch (4) x Megatron tensor-
parallel over heads (2 ranks x 8 heads). Each core computes a full-output
partial for its batch; host sums the two rank partials per batch.

v2 design (single-pass pipelined, bf16 data path):
  - All SBUF data is bf16 (PSUM stays fp32): DVE elementwise ops run in 2x
    mode, weight loads get FWL (128-col bf16 stationaries), DMA halves.
  - One fused schedule: token-block tb loop projects q/k/v for tb and
    interleaves the ACT-heavy attention of query-block tb-? between PE work,
    so exp overlaps projections instead of running as a separate phase.
  - Scores for the two heads of a pair are issued back-to-back with base
    partitions 0/64 -> the PE runs them concurrently (row-group tiling).
  - The v stationary carries a 64-wide ones block (M=128): the AV matmul
    produces softmax denominators replicated on psum partitions 64:127 for
    free, so normalization is a [64,512] reciprocal + one multiply.
  - One exp per (pair, kt-pair) over a [128,2048] psum tile (4 banks).
  - q/k head features are pre-permuted on the host ([even|odd] per head) so
    interleaved RoPE becomes rotate-half. Softmax skips max-subtraction
    (|s|/8 < 40 for this distribution, safe in fp32/bf16 exponent range).
"""
import numpy as np

B, T, D = 4, 2048, 1024
H_TOTAL, HD = 16, 64
N_CORES = 8
H = H_TOTAL // 2        # heads per core (TP rank)
NP = H // 2             # head pairs per core
FS = H * HD             # 512 sharded q/k/v features per core
DC = D // 128           # 8 d_model chunks
TB = T // 512           # 4 token blocks (query blocks)
TC = T // 128           # 16 token chunks (key tiles)
SCALE = 1.0 / np.sqrt(HD)

_CACHE = {}


def _split_waits(nc, mybir, maxw=1):
    """This env's walrus encodes at most one sem wait per instruction; move
    extra waits onto same-engine NoOp carriers inserted just before."""
    import copy
    eng_map = {
        mybir.EngineType.PE: nc.tensor,
        mybir.EngineType.DVE: nc.vector,
        mybir.EngineType.Activation: nc.scalar,
        mybir.EngineType.Pool: nc.gpsimd,
        mybir.EngineType.SP: nc.sync,
    }
    protos = {}

    def proto(engine):
        if engine not in protos:
            mi = eng_map[engine].nop(nofuse=True).ins
            for blk in nc.m.functions[0].blocks:
                insts = list(blk.instructions)
                if insts and insts[-1].name == mi.name:
                    blk.instructions = insts[:-1]
                    break
            protos[engine] = mi
        return protos[engine]

    ctr = 0
    for blk in nc.m.functions[0].blocks:
        out = []
        changed = False
        for inst in blk.instructions:
            si = inst.sync_info
            waits = list(si.on_wait) if si and si.on_wait else []
            if len(waits) > maxw and getattr(inst, "engine", None) is not None:
                head, keep = waits[:-maxw], waits[-maxw:]
                for i in range(0, len(head), maxw):
                    nop = copy.deepcopy(proto(inst.engine))
                    ctr += 1
                    nop.name = f"I-wsplit-{ctr}"
                    nop.sync_info = mybir.SyncInfo(on_wait=head[i:i + maxw], on_update=[])
                    out.append(nop)
                si.on_wait = keep
                changed = True
            out.append(inst)
        if changed:
            blk.instructions = out
    return nc


def _build_nc(R=1, attn=True):
    import concourse.bass as bass
    import concourse.mybir as mybir
    import concourse.tile as tile

    f32 = mybir.dt.float32
    bf16 = mybir.dt.bfloat16
    Exp = mybir.ActivationFunctionType.Exp

    nc = bass.Bass("TRN2", target_bir_lowering=False, debug=False)
    xT = nc.dram_tensor("xT", [D, T], bf16, kind="ExternalInput").ap()
    wqT = nc.dram_tensor("wqT", [D, FS], bf16, kind="ExternalInput").ap()
    wkT = nc.dram_tensor("wkT", [D, FS], bf16, kind="ExternalInput").ap()
    wvT = nc.dram_tensor("wvT", [D, FS], bf16, kind="ExternalInput").ap()
    woT = nc.dram_tensor("woT", [FS, D], bf16, kind="ExternalInput").ap()
    ropeC = nc.dram_tensor("ropeC", [128, T], bf16, kind="ExternalInput").ap()
    ropeS = nc.dram_tensor("ropeS", [128, T], bf16, kind="ExternalInput").ap()
    masks = nc.dram_tensor("masks", [4, 128, 512], bf16, kind="ExternalInput").ap()
    out = nc.dram_tensor("out", [T, D], f32, kind="ExternalOutput").ap()

    with tile.TileContext(nc) as tc:
      for _rep in range(R):
        with tc.tile_pool(name="persist", bufs=1) as persist, \
             tc.tile_pool(name="pp", bufs=2, space="PSUM") as pp, \
             tc.tile_pool(name="pss", bufs=2, space="PSUM") as pss, \
             tc.tile_pool(name="psc", bufs=1, space="PSUM") as psc, \
             tc.tile_pool(name="attn", bufs=6) as attnp, \
             tc.tile_pool(name="ptmp", bufs=2) as ptmp, \
             tc.tile_pool(name="nrm", bufs=2) as nrm, \
             tc.tile_pool(name="obuf", bufs=4) as obuf:
            xT_s = persist.tile([128, DC, T], bf16)
            qT = persist.tile([128, NP, T], bf16)
            kT = persist.tile([128, NP, T], bf16)
            vON = persist.tile([128, TC, H, 128], bf16)  # [tok, kt, h, v|ones]
            ctxT = persist.tile([128, NP, T], bf16)
            wqs = persist.tile([128, DC, FS], bf16)
            wks = persist.tile([128, DC, FS], bf16)
            wvs = persist.tile([128, DC, FS], bf16)
            wos = persist.tile([128, NP, D], bf16)
            rc = persist.tile([128, T], bf16)
            rs = persist.tile([128, T], bf16)
            mk = persist.tile([128, 4, 512], bf16)

            xr = xT.rearrange("(c p) t -> p c t", p=128)
            nc.sync.dma_start(xT_s[:, :, 0:512], xr[:, :, 0:512])
            nc.sync.dma_start(wks, wkT.rearrange("(c p) m -> p c m", p=128))
            nc.sync.dma_start(wqs, wqT.rearrange("(c p) m -> p c m", p=128))
            nc.sync.dma_start(rc, ropeC)
            nc.sync.dma_start(rs, ropeS)
            nc.sync.dma_start(wvs, wvT.rearrange("(c p) m -> p c m", p=128))
            for tb in range(1, TB):
                nc.sync.dma_start(
                    xT_s[:, :, tb * 512:(tb + 1) * 512],
                    xr[:, :, tb * 512:(tb + 1) * 512],
                )
            nc.sync.dma_start(mk, masks.rearrange("j p f -> p j f"))
            nc.sync.dma_start(wos, woT.rearrange("(c p) o -> p c o", p=128))
            # ones block of the AV stationary (denominator trick)
            nc.gpsimd.memset(vON[:, :, :, HD:128], 1.0)
            if attn in ("noact", "expfree"):
                at_dummy = persist.tile([128, 1024], bf16)
                nc.vector.memset(at_dummy, 0.001)
                nc.vector.memset(ctxT[:, :, :], 0.5)

            def proj_group(dst, w_sb, pair, tb):
                tsl = slice(tb * 512, (tb + 1) * 512)
                psum = pp.tile([128, 512], f32, tag="p512")
                for kc in range(DC):
                    nc.tensor.matmul(
                        psum,
                        w_sb[:, kc, pair * 128:(pair + 1) * 128],
                        xT_s[:, kc, tsl],
                        start=(kc == 0), stop=(kc == DC - 1),
                    )
                dsl = dst[:, pair, tsl]
                # DVE evict: keeps the ACT stream pure-exp during attention
                nc.vector.tensor_copy(out=dsl, in_=psum)
                # rotate-half rope; in-place cos-mul after the shifted
                # sin-muls read the raw values (WAR dep).
                tmp = ptmp.tile([128, 512], bf16, tag="ropetmp")
                for eng, hb in ((nc.vector, 0), (nc.gpsimd, 64)):
                    eng.tensor_mul(
                        out=tmp[hb:hb + 32, :],
                        in0=dsl[hb + 32:hb + 64],
                        in1=rs[hb + 32:hb + 64, tsl],
                    )
                    eng.tensor_mul(
                        out=tmp[hb + 32:hb + 64, :],
                        in0=dsl[hb:hb + 32],
                        in1=rs[hb:hb + 32, tsl],
                    )
                nc.vector.tensor_mul(out=dsl, in0=dsl, in1=rc[:, tsl])
                nc.vector.tensor_add(out=dsl, in0=dsl, in1=tmp)

            def v_group(tcv):
                psum = pp.tile([128, 512], f32, tag="p512")
                for kc in range(DC):
                    nc.tensor.matmul(
                        psum,
                        xT_s[:, kc, tcv * 128:(tcv + 1) * 128],
                        wvs[:, kc, :],
                        start=(kc == 0), stop=(kc == DC - 1),
                    )
                nc.vector.tensor_copy(
                    out=vON[:, tcv, :, 0:HD],
                    in_=psum.rearrange("p (h e) -> p h e", e=HD),
                )

            def outproj_chunk(qb, i):
                # both 512-wide output halves of one token chunk: the second
                # matmul of each fc reuses the loaded stationary (no LDWEIGHTS)
                tco = 4 * qb + i
                psA = pp.tile([128, 512], f32, tag="p512")
                psB = pp.tile([128, 512], f32, tag="p512")
                for fc in range(NP):
                    for ps, osl in ((psA, slice(0, 512)), (psB, slice(512, 1024))):
                        mi = nc.tensor.matmul(
                            ps,
                            ctxT[:, fc, tco * 128:(tco + 1) * 128],
                            wos[:, fc, osl],
                            start=(fc == 0), stop=(fc == NP - 1),
                        )
                        if osl.start:
                            mi.ins.ldweights = False
                for ps, osl in ((psA, slice(0, 512)), (psB, slice(512, 1024))):
                    ot = obuf.tile([128, 512], f32, tag="ot")
                    nc.vector.tensor_copy(out=ot, in_=ps)
                    nc.sync.dma_start(out[tco * 128:(tco + 1) * 128, osl], ot)

            def tb_groups(tb):
                gs = []
                for pair in range(NP):
                    gs.append(lambda p=pair, t=tb: proj_group(kT, wks, p, t))
                for pair in range(NP):
                    gs.append(lambda p=pair, t=tb: proj_group(qT, wqs, p, t))
                for tcv in range(4 * tb, 4 * tb + 4):
                    gs.append(lambda c=tcv: v_group(c))
                return gs

            filler = []

            def emit_filler(n):
                for _ in range(n):
                    if filler:
                        filler.pop(0)()

            # startup: project everything for tb=0 directly
            for g in tb_groups(0):
                g()

            if not attn:
                # timing-bisect mode: skip attention, fake ctxT
                nc.vector.memset(ctxT[:, :, :], 0.5)
                for tb in range(1, TB):
                    for g in tb_groups(tb):
                        g()
                for qb in range(TB):
                    for i in range(4):
                        outproj_chunk(qb, i)

            for qb in range(TB if attn else 0):
                if qb + 1 < TB:
                    filler.extend(tb_groups(qb + 1))
                if qb > 0:
                    filler.extend(
                        lambda q=qb - 1, j=i: outproj_chunk(q, j) for i in range(4)
                    )
                qsl = slice(qb * 512, (qb + 1) * 512)
                nkt = 4 * qb + 4
                for pair in range(NP):
                    pctx = {}
                    for hi in (0, 1):
                        pctx_h = psc.tile([128, 512], f32, tag=f"pctx{hi}")
                        pctx[hi] = pctx_h

                    def do_mask(kt, at):
                        # causal mask varies only in the 128-wide diagonal
                        # strip; [0:128j) was zeroed on Pool
                        j = kt - 4 * qb
                        if j < 0 or attn == "noact":
                            return
                        for hi in (0, 1):
                            st = hi * 512 + 128 * j
                            nc.vector.tensor_mul(
                                out=at[:, st:st + 128], in0=at[:, st:st + 128],
                                in1=mk[:, j, 128 * j:128 * j + 128],
                            )

                    def do_av(kt, at):
                        av_src = at_dummy if attn == "expfree" else at
                        for hi in (0, 1):
                            sl = slice(hi * 512, (hi + 1) * 512)
                            nc.tensor.matmul(
                                pctx[hi],
                                vON[:, kt, 2 * pair + hi, :],
                                av_src[:, sl],
                                start=(kt == 0), stop=(kt == nkt - 1),
                            )

                    pend = []
                    for kt in range(nkt):
                        ps = pss.tile([128, 1024], f32, tag="ps")
                        at = attnp.tile([128, 1024], bf16, tag="at")
                        if attn == "noact":
                            at = at_dummy
                        j = kt - 4 * qb
                        full_exp = j <= 0 or (qb == 0 and pair == 0 and kt < 3)
                        for hi, po in ((0, 0), (1, 64)):
                            nc.tensor.matmul(
                                ps[:, hi * 512:(hi + 1) * 512],
                                kT[po:po + 64, pair, kt * 128:(kt + 1) * 128],
                                qT[po:po + 64, pair, qsl],
                                start=True, stop=True,
                            )
                        # diagonal tiles: only columns f >= 128j can be unmasked;
                        # exp just that band (mask-mul later zeroes the stale rest).
                        # The very first at-buffer uses get full width so no
                        # uninitialized SBUF (potential NaN) survives the mask mul.
                        if attn == "noact":
                            pass
                        elif not full_exp:
                            nc.scalar.activation(
                                at.rearrange("p (h f) -> p h f", h=2)[:, :, 128 * j:],
                                ps.rearrange("p (h f) -> p h f", h=2)[:, :, 128 * j:],
                                Exp, scale=float(SCALE),
                            )
                        else:
                            nc.scalar.activation(at, ps, Exp, scale=float(SCALE))
                        if j > 0 and attn != "noact":
                            # zero the always-masked [0:128j) bands (Pool, off
                            # the exp->AV critical chain)
                            nc.gpsimd.memset(
                                at.rearrange("p (h f) -> p h f", h=2)[:, :, 0:128 * j],
                                0.0,
                            )
                        # mask lags exp by 1 unit, AV by 2: the mask gets a
                        # full unit-period of DVE queue turn before its AV,
                        # and `at` is ready when the in-order PE queue
                        # reaches the AV matmuls
                        pend.append((kt, at))
                        if len(pend) >= 2:
                            do_mask(*pend[-2])
                        if len(pend) >= 3:
                            do_av(*pend.pop(0))
                        if kt % 2 == 1:
                            emit_filler(1)
                    do_mask(*pend[-1])
                    for p in pend:
                        do_av(*p)
                    pend.clear()
                    for hi, po in ((0, 0), (1, 64)):
                        if attn == "noact":
                            continue
                        rinv = nrm.tile([64, 512], f32, tag=f"rinv{hi}")
                        with nc.allow_low_precision(reason="softmax denominator"):
                            nc.vector.reciprocal(rinv, pctx[hi][HD:128, :])
                        nc.vector.tensor_mul(
                            out=ctxT[po:po + 64, pair, qsl],
                            in0=pctx[hi][0:HD, :],
                            in1=rinv,
                        )
                emit_filler(len(filler))
            if attn:
                for i in range(4):
                    outproj_chunk(TB - 1, i)

    _split_waits(nc, mybir)
    return nc


def _make_runner(nc, n_cores):
    """Build the shard_map-jitted PJRT executable once (reusable across calls)."""
    import jax
    import concourse.mybir as mybir
    from jax.sharding import Mesh, PartitionSpec
    from jax.experimental.shard_map import shard_map
    from concourse import bass2jax as b2j

    b2j.install_neuronx_cc_hook()
    partition_name = nc.partition_id_tensor.name if nc.partition_id_tensor else None
    in_names, out_names, out_avals = [], [], []
    for alloc in nc.m.functions[0].allocations:
        if not isinstance(alloc, mybir.MemoryLocationSet):
            continue
        name = alloc.memorylocations[0].name
        if alloc.kind == "ExternalInput":
            if name != partition_name:
                in_names.append(name)
        elif alloc.kind == "ExternalOutput":
            out_names.append(name)
            out_avals.append(
                jax.core.ShapedArray(tuple(alloc.tensor_shape), mybir.dt.np(alloc.dtype))
            )
    all_in_names = list(in_names) + list(out_names)
    if partition_name is not None:
        all_in_names.append(partition_name)

    def _body(*args):
        operands = list(args)
        if partition_name is not None:
            operands.append(b2j.partition_id_tensor())
        return tuple(b2j._bass_exec_p.bind(
            *operands,
            out_avals=tuple(out_avals),
            in_names=tuple(all_in_names),
            out_names=tuple(out_names),
            lowering_input_output_aliases=(),
            sim_require_finite=True,
            sim_require_nnan=True,
            nc=nc,
        ))

    devices = jax.devices()[:n_cores]
    mesh = Mesh(np.asarray(devices), ("core",))
    n_in = len(in_names) + len(out_names)
    fn = jax.jit(
        shard_map(
            _body, mesh=mesh,
            in_specs=(PartitionSpec("core"),) * n_in,
            out_specs=(PartitionSpec("core"),) * len(out_names),
            check_rep=False,
        ),
        keep_unused=True,
    )



    def stage(in_maps):
        import jax as _jax
        per_core = [[np.asarray(m[name]) for name in in_names] for m in in_maps]
        concat_in = [
            np.concatenate([per_core[c][i] for c in range(n_cores)], axis=0)
            for i in range(len(in_names))
        ]
        concat_zeros = [
            np.zeros((n_cores * a.shape[0], *a.shape[1:]), a.dtype) for a in out_avals
        ]
        return [_jax.device_put(a) for a in concat_in + concat_zeros]

    def call_staged(staged):
        import jax as _jax
        out_arrs = fn(*staged)
        _jax.block_until_ready(out_arrs)
        return out_arrs

    def time_staged(staged):
        return call_staged(staged)

    def call(in_maps):
        import jax as _jax
        out_arrs = call_staged(stage(in_maps))
        return [
            {name: np.asarray(out_arrs[i]).reshape(n_cores, *out_avals[i].shape)[c]
             for i, name in enumerate(out_names)}
            for c in range(n_cores)
        ]

    call.stage = stage
    call.call_staged = call_staged
    call.time_staged = time_staged
    return call


def _host_tables():
    import ml_dtypes
    bf16 = ml_dtypes.bfloat16
    # rope tables in the permuted ([even dims | odd dims] per head) layout:
    # rows 0:32 -> freq j (x1 of head A), 32:64 -> freq j (x2 of head A), repeat.
    j = np.arange(32, dtype=np.float64)
    inv = 1.0 / (10000.0 ** (2.0 * j / HD))
    t = np.arange(T, dtype=np.float64)
    ang = np.outer(inv, t)                      # [32, T]
    c32 = np.cos(ang).astype(np.float32)
    s32 = np.sin(ang).astype(np.float32)
    ropeC = np.concatenate([c32, c32, c32, c32], axis=0).astype(bf16)    # [128, T]
    ropeS = np.concatenate([s32, -s32, s32, -s32], axis=0).astype(bf16)  # [128, T]
    # causal masks for diagonal tiles: mask[j][p, f] = 1 if p <= f - 128j
    p = np.arange(128)[:, None]
    f = np.arange(512)[None, :]
    masks = np.stack(
        [(p <= f - 128 * jj).astype(np.float32) for jj in range(4)], axis=0
    ).astype(bf16)
    return ropeC, ropeS, masks


def _perm_rows():
    # per head: [even dims, odd dims]
    perm = []
    for h in range(H):
        base = h * HD
        perm.extend(base + np.arange(0, HD, 2))
        perm.extend(base + np.arange(1, HD, 2))
    return np.asarray(perm)


def _prep_in_maps(x, Wq, Wk, Wv, Wo):
    import ml_dtypes
    bf16 = ml_dtypes.bfloat16
    ropeC, ropeS, masks = _host_tables()
    perm = _perm_rows()
    in_maps = []
    for c in range(N_CORES):
        b, r = c // 2, c % 2
        rows = slice(r * FS, (r + 1) * FS)
        in_maps.append({
            "xT": np.ascontiguousarray(x[b].T).astype(bf16),
            "wqT": np.ascontiguousarray(Wq[rows][perm].T).astype(bf16),
            "wkT": np.ascontiguousarray(Wk[rows][perm].T).astype(bf16),
            "wvT": np.ascontiguousarray(Wv[rows].T).astype(bf16),
            "woT": np.ascontiguousarray(Wo[:, rows].T).astype(bf16),
            "ropeC": ropeC,
            "ropeS": ropeS,
            "masks": masks,
        })
    return in_maps


def kernel(x, Wq, Wk, Wv, Wo):
    x = np.asarray(x, dtype=np.float32)
    Wq = np.asarray(Wq, dtype=np.float32)
    Wk = np.asarray(Wk, dtype=np.float32)
    Wv = np.asarray(Wv, dtype=np.float32)
    Wo = np.asarray(Wo, dtype=np.float32)

    if "runner" not in _CACHE:
        nc = _build_nc()
        _CACHE["runner"] = _make_runner(nc, N_CORES)
    call = _CACHE["runner"]

    results = call(_prep_in_maps(x, Wq, Wk, Wv, Wo))
    out = np.empty((B, T, D), dtype=np.float32)
    for b in range(B):
        out[b] = results[2 * b]["out"] + results[2 * b + 1]["out"]
    return out
